# revision 1
# baseline (speedup 1.0000x reference)
"""Trainium2 Bass kernel for 3D neighborhood attention (sparse_attention).

Problem: q,k [1,40,40,40,48] fp32, rpb [8,3,3,3]; out [1,24,40,40,40].
Per voxel x: logits[h,kk] = scale * <q[x,h,:], k[x+off_kk,h,:]> + rpb[h,kk]
(zero-padded k at boundaries, kk over 3x3x3 offsets), p = softmax over kk,
out[x,h,:] = sum_kk p[h,kk] * off_kk  (constant integer offsets as values).

Sharding: spatial-parallel over H (40 -> 8 slabs of 5). Each core gets its
q slab plus a host-side im2col of the 27 shifted k views for its slab
(halo handled on host), so on-core everything is token-parallel with
tokens on SBUF partitions (2 tokens per partition) and no cross-partition
data movement. The PV contraction exploits that the "values" are the
constant offsets in {-1,0,1}^3: out_i = (sum of exp over di=+1 block) -
(sum over di=-1 block), so it is pure block reductions, no multiplies.
"""

import numpy as np

import concourse.bass as bass
import concourse.tile as tile
from concourse import bacc, mybir
from concourse.bass_utils import run_bass_kernel_spmd

NH = 8
HD = 6
DIM = NH * HD
KS = 3
NT = KS**3  # 27
SCALE = HD**-0.5
H = W = T = 40
N_CORES = 8
SLAB = H // N_CORES          # 5 rows of H per core
TOK = SLAB * W * T           # 8000 tokens per core
P = 128
TPP = 2                      # tokens per partition
TILES = 32                   # ceil(8000 / 256)
TOKP = TILES * P * TPP       # 8192
FKC = NT * DIM               # 1296  (kk, c) free dim per token
FKH = NT * NH                # 216   (kk, h) free dim per token

_prog_cache = {}


def _build_program():
    fp32 = mybir.dt.float32
    nc = bacc.Bacc("TRN2", target_bir_lowering=False, debug=False,
                   num_devices=N_CORES)
    qs = nc.dram_tensor("qs", [TILES, P, TPP * DIM], fp32,
                        kind="ExternalInput").ap()
    kn = nc.dram_tensor("kn", [TILES, P, TPP * FKC], fp32,
                        kind="ExternalInput").ap()
    rpbt = nc.dram_tensor("rpbt", [P, FKH], fp32, kind="ExternalInput").ap()
    out = nc.dram_tensor("out", [TILES, P, TPP * 3 * NH], fp32,
                         kind="ExternalOutput").ap()

    X = mybir.AxisListType.X
    XY = mybir.AxisListType.XY
    ADD = mybir.AluOpType.add

    with tile.TileContext(nc) as tc:
        with (
            tc.tile_pool(name="consts", bufs=1) as cpool,
            tc.tile_pool(name="kin", bufs=3) as kpool,
            tc.tile_pool(name="qin", bufs=3) as qpool,
            tc.tile_pool(name="prod", bufs=2) as ppool,
            tc.tile_pool(name="logit", bufs=3) as lpool,
            tc.tile_pool(name="expv", bufs=3) as epool,
            tc.tile_pool(name="small", bufs=16) as spool,
            tc.tile_pool(name="outp", bufs=3) as opool,
        ):
            rpb_sb = cpool.tile([P, FKH], fp32)
            nc.sync.dma_start(rpb_sb[:], rpbt[:])

            for ti in range(TILES):
                kt = kpool.tile([P, TPP * FKC], fp32)
                nc.sync.dma_start(kt[:], kn[ti])
                qt = qpool.tile([P, TPP * DIM], fp32)
                nc.sync.dma_start(qt[:], qs[ti])

                # P4[p, j, kk, c] = kn[p, j, kk, c] * q[p, j, c]
                pt = ppool.tile([P, TPP * FKC], fp32)
                q_b = (qt[:].rearrange("p (j c) -> p j c", j=TPP)
                       .unsqueeze(2).broadcast_to([P, TPP, NT, DIM]))
                nc.vector.tensor_mul(
                    pt[:].rearrange("p (j kk c) -> p j kk c", j=TPP, kk=NT),
                    kt[:].rearrange("p (j kk c) -> p j kk c", j=TPP, kk=NT),
                    q_b,
                )
                # L[p, (j,kk,h)] = sum_d P4[p, j, (kk,h), d]
                lt = lpool.tile([P, TPP * FKH], fp32)
                nc.vector.tensor_reduce(
                    lt[:],
                    pt[:].rearrange("p (j kh d) -> p j kh d", j=TPP, d=HD),
                    axis=X, op=ADD,
                )
                # L2 = L + rpb  (q was pre-scaled by SCALE on host)
                l2 = lpool.tile([P, TPP * FKH], fp32)
                rpb_b = rpb_sb[:].unsqueeze(1).broadcast_to([P, TPP, FKH])
                nc.vector.tensor_add(
                    l2[:].rearrange("p (j f) -> p j f", j=TPP),
                    lt[:].rearrange("p (j f) -> p j f", j=TPP),
                    rpb_b,
                )
                # E = exp(L2)  (ScalarE, overlaps with DVE)
                et = epool.tile([P, TPP * FKH], fp32)
                nc.scalar.activation(et[:], l2[:],
                                     mybir.ActivationFunctionType.Exp)

                # Softmax denominator: S0[p, (j,h)] = sum_kk E
                e_khk = et[:].rearrange("p (j kk h) -> p j h kk",
                                        j=TPP, kk=NT, h=NH)
                s0 = spool.tile([P, TPP * NH], fp32)
                nc.vector.tensor_reduce(s0[:], e_khk, axis=X, op=ADD)

                # Directional numerators via paired block sums over the
                # +-1 slabs of each axis (values are +-1/0).
                # E free layout: (j, di, dj, dl, h).  V layout: (o, j, pm, h)
                v_di = et[:].rearrange(
                    "p (j di dj dl h) -> p j di h (dj dl)",
                    j=TPP, di=KS, dj=KS, dl=KS, h=NH)
                v_dj = et[:].rearrange(
                    "p (j di dj dl h) -> p j dj h di dl",
                    j=TPP, di=KS, dj=KS, dl=KS, h=NH)
                v_dl = et[:].rearrange(
                    "p (j di dj dl h) -> p j dl h di dj",
                    j=TPP, di=KS, dj=KS, dl=KS, h=NH)

                vt = spool.tile([P, 3 * 2 * TPP * NH], fp32)  # [128, 96]
                npm = TPP * NH
                for o, (v, ax) in enumerate(((v_di, X), (v_dj, XY),
                                             (v_dl, XY))):
                    for pm in range(2):
                        nc.vector.tensor_reduce(
                            vt[:, (o * 2 + pm) * npm:(o * 2 + pm + 1) * npm],
                            v[:, :, 2 * pm], axis=ax, op=ADD)

                # S3[p, (o,j,h)] = V[.., pm=1] - V[.., pm=0]
                v5 = vt[:].rearrange("p (o pm j h) -> p o pm j h",
                                     o=3, pm=2, j=TPP)
                s3 = spool.tile([P, 3 * TPP * NH], fp32)
                nc.vector.tensor_sub(
                    s3[:].rearrange("p (o j h) -> p o j h", o=3, j=TPP),
                    v5[:, :, 1], v5[:, :, 0])

                rt = spool.tile([P, TPP * NH], fp32)
                nc.vector.reciprocal(rt[:], s0[:])
                # out[p, (o,j,h)] = S3 * (1/S0)
                ot = opool.tile([P, TPP * 3 * NH], fp32)
                r_b = (rt[:].rearrange("p (j h) -> p j h", j=TPP)
                       .unsqueeze(1).broadcast_to([P, 3, TPP, NH]))
                nc.vector.tensor_mul(
                    ot[:].rearrange("p (o j h) -> p o j h", o=3, j=TPP),
                    s3[:].rearrange("p (o j h) -> p o j h", o=3, j=TPP),
                    r_b)
                nc.sync.dma_start(out[ti], ot[:])

    nc.compile()
    return nc


def _host_prep(q, k, rpb):
    q = np.asarray(q, dtype=np.float32)
    k = np.asarray(k, dtype=np.float32)
    rpb = np.asarray(rpb, dtype=np.float32)

    q0 = (q[0] * SCALE).astype(np.float32)          # [40,40,40,48]
    kp = np.pad(k[0], ((1, 1), (1, 1), (1, 1), (0, 0)))  # [42,42,42,48]
    win = np.lib.stride_tricks.sliding_window_view(kp, (KS, KS, KS),
                                                   axis=(0, 1, 2))
    # win: [40,40,40,48,3,3,3] -> [40,40,40,(kk,c)]
    win = np.ascontiguousarray(win.transpose(0, 1, 2, 4, 5, 6, 3))
    win = win.reshape(H, W, T, FKC)

    rpb_kh = np.ascontiguousarray(rpb.reshape(NH, NT).T).reshape(FKH)
    rpb_t = np.broadcast_to(rpb_kh, (P, FKH)).copy()

    in_maps = []
    for i in range(N_CORES):
        h0 = i * SLAB
        q_pad = np.zeros((TOKP, DIM), np.float32)
        q_pad[:TOK] = q0[h0:h0 + SLAB].reshape(TOK, DIM)
        kn_pad = np.zeros((TOKP, FKC), np.float32)
        kn_pad[:TOK] = win[h0:h0 + SLAB].reshape(TOK, FKC)
        in_maps.append({
            "qs": q_pad.reshape(TILES, P, TPP * DIM),
            "kn": kn_pad.reshape(TILES, P, TPP * FKC),
            "rpbt": rpb_t,
        })
    return in_maps


def _assemble(results):
    slabs = []
    for i in range(N_CORES):
        o = results[i]["out"].reshape(TILES, P, 3, TPP, NH)
        o = o.transpose(0, 1, 3, 2, 4).reshape(TOKP, 3, NH)[:TOK]
        o = o.reshape(SLAB, W, T, 3, NH)
        # channel order in reference: c = h*3 + o
        slabs.append(o.transpose(0, 1, 2, 4, 3).reshape(SLAB, W, T, 3 * NH))
    full = np.concatenate(slabs, axis=0)             # [40,40,40,24]
    return np.ascontiguousarray(full.transpose(3, 0, 1, 2))[None]


def _run(q, k, rpb, **spmd_kwargs):
    if "prog" not in _prog_cache:
        _prog_cache["prog"] = _build_program()
    nc = _prog_cache["prog"]
    in_maps = _host_prep(q, k, rpb)
    res = run_bass_kernel_spmd(nc, in_maps, list(range(N_CORES)),
                               **spmd_kwargs)
    return _assemble(res.results), res


def kernel(q, k, rpb):
    out, _ = _run(q, k, rpb)
    return out



# revision 6
# speedup vs baseline: 1.1931x; 1.1931x over previous
"""Trainium2 Bass kernel for 3D neighborhood attention (sparse_attention).

Problem: q,k [1,40,40,40,48] fp32, rpb [8,3,3,3]; out [1,24,40,40,40].
Per voxel x: logits[h,kk] = scale * <q[x,h,:], k[x+off_kk,h,:]> + rpb[h,kk]
(zero-padded k at boundaries, kk over 3x3x3 offsets), p = softmax over kk,
out[x,h,:] = sum_kk p[h,kk] * off_kk  (constant integer offsets as values).

Sharding: spatial-parallel over H (40 -> 8 slabs of 5 i-rows per core).

On-core layout (all fp16): partitions p = j*3 + lc (40 j x 3 l-chunks =
120); free axes carry (i, l-within-chunk, channel).  Channels are stored
d-major (c' = d*8 + h) so the head dim d is sliceable at stride 8 with
heads packed contiguously (innermost stride-1 runs keep the DVE 4x_2p
perf mode engaged).  The 27 neighbor offsets decompose as:
  dj (partition shift) -> 3 separate k tiles DMA'd from row-shifted
      slices of one padded HBM copy (engine operands cannot start at
      arbitrary partitions, DMA can);
  di (i shift)         -> free-dim offset (k holds a 7-row i halo);
  dl (l shift)         -> free-dim offset (each l-chunk holds a 16-wide
      halo'd l window).
Products and all reductions are scalar_tensor_tensor ops (4x perf mode);
tensor_reduce (always 1x) is never used.  exp runs on the Act engine.
The PV contraction uses the constant-offset structure: out_o =
(sum_{axis o = +1} p - sum_{axis o = -1} p) / sum p, i.e. pairwise adds.
A -2.0 logit shift is folded into rpb so fp16 exp cannot overflow.
"""

import numpy as np

import concourse.bass as bass
import concourse.tile as tile
from concourse import bacc, mybir
from concourse.ap import AP
from concourse.bass_utils import run_bass_kernel_spmd

NH = 8
HD = 6
DIM = NH * HD
KS = 3
SCALE = HD**-0.5
SHIFT = 2.0                  # folded into rpb; cancels in softmax
H = W = T = 40
N_CORES = 8
SLAB = H // N_CORES          # 5 i-rows per core
NLC = 3                      # l-chunks per j
LCH = 14                     # l per chunk (last chunk: 12 valid + 2 pad)
P_OUT = W * NLC              # 120 partitions
P_K = (W + 2) * NLC          # 126 rows in the padded k HBM array
ILH = SLAB * LCH             # 70 (i, l) token slots per partition
QF = SLAB * LCH * DIM        # 3360 q free size
KI = SLAB + 2                # 7 i rows in k tile
KL = LCH + 2                 # 16 l slots in k tile
KF = KI * KL * DIM           # 5376 k free size
LF = 27 * ILH * NH           # 15120 logits free size
OF = 3 * ILH * NH            # 1680 out free size

MULT = mybir.AluOpType.mult
ADD = mybir.AluOpType.add
SUB = mybir.AluOpType.subtract

_prog_cache = {}


def _ap(base, offset, dims):
    """View of tile AP `base` at free-elem `offset` with free dims
    [(stride, count), ...]; keeps the full partition dim."""
    return AP(base.tensor, base.offset + offset,
              [list(base.ap[0])] + [list(d) for d in dims])


def _build_program():
    fp16 = mybir.dt.float16
    fp32 = mybir.dt.float32
    nc = bacc.Bacc("TRN2", target_bir_lowering=False, debug=False,
                   num_devices=N_CORES)
    qd = nc.dram_tensor("qd", [P_OUT, QF], fp16, kind="ExternalInput").ap()
    kd = nc.dram_tensor("kd", [P_K, KF], fp16, kind="ExternalInput").ap()
    rd = nc.dram_tensor("rd", [P_OUT, 27 * NH], fp16,
                        kind="ExternalInput").ap()
    od = nc.dram_tensor("od", [P_OUT, OF], fp16, kind="ExternalOutput").ap()

    with tile.TileContext(nc) as tc:
        with (
            tc.tile_pool(name="io", bufs=1) as io,
            tc.tile_pool(name="work", bufs=1) as wk,
        ):
            rpb = io.tile([P_OUT, 27 * NH], fp16)
            nc.sync.dma_start(rpb[:], rd[:])
            q = io.tile([P_OUT, QF], fp16)
            for c in range(2):
                nc.sync.dma_start(q[:, c * 1680:(c + 1) * 1680],
                                  qd[:, c * 1680:(c + 1) * 1680])
            kt = []
            for dj in range(3):
                t = io.tile([P_OUT, KF], fp16, name=f"kt{dj}")
                for c in range(4):
                    nc.sync.dma_start(
                        t[:, c * 1344:(c + 1) * 1344],
                        kd[3 * dj:3 * dj + P_OUT, c * 1344:(c + 1) * 1344])
                kt.append(t)

            L = wk.tile([P_OUT, LF], fp16)   # (dj, di, dl, il, h)
            E = wk.tile([P_OUT, LF], fp16)
            Sdj = []                          # per-dj sums over (di, d)
            Tdj = []                          # per-dj sums over (di, dl, d)

            for dj in range(3):
                qv = _ap(q[:], 0, [(672, 5), (48, 14), (1, 48)])
                for di in range(3):
                    for dl in range(3):
                        # P4[il, (d,h)] = q[il, (d,h)] * k[il + (di,dj,dl)]
                        # (TensorScalarPtr operands must canonicalize to
                        # <=3 AP dims, so one instruction per offset)
                        P4 = wk.tile([P_OUT, ILH * DIM], fp16)
                        kv = _ap(kt[dj][:], di * (KL * DIM) + dl * DIM,
                                 [(KL * DIM, 5), (48, 14), (1, 48)])
                        nc.vector.scalar_tensor_tensor(P4[:], qv, 1.0, kv,
                                                       MULT, MULT)
                        # d-reduction: 6 -> 3 -> (+rpb) -> 1
                        A = wk.tile([P_OUT, ILH * 3 * NH], fp16)
                        av = _ap(A[:], 0, [(24, 70), (8, 3), (1, 8)])
                        p0 = _ap(P4[:], 0, [(48, 70), (8, 3), (1, 8)])
                        p1 = _ap(P4[:], 24, [(48, 70), (8, 3), (1, 8)])
                        nc.vector.scalar_tensor_tensor(av, p0, 1.0, p1,
                                                       MULT, ADD)
                        A2 = wk.tile([P_OUT, ILH * NH], fp16)
                        a0 = _ap(A[:], 0, [(24, 70), (1, 8)])
                        a1 = _ap(A[:], 8, [(24, 70), (1, 8)])
                        nc.vector.scalar_tensor_tensor(A2[:], a0, 1.0, a1,
                                                       MULT, ADD)
                        A3 = wk.tile([P_OUT, ILH * NH], fp16)
                        a2 = _ap(A[:], 16, [(24, 70), (1, 8)])
                        rv = _ap(rpb[:], di * 72 + dj * 24 + dl * 8,
                                 [(0, 70), (1, 8)])
                        nc.vector.scalar_tensor_tensor(A3[:], a2, 1.0, rv,
                                                       MULT, ADD)
                        nc.vector.scalar_tensor_tensor(
                            L[:, dj * 5040 + di * 1680 + dl * 560:
                              dj * 5040 + di * 1680 + (dl + 1) * 560],
                            A2[:], 1.0, A3[:], MULT, ADD)
                # exp of this dj block on the Act engine
                nc.scalar.activation(E[:, dj * 5040:(dj + 1) * 5040],
                                     L[:, dj * 5040:(dj + 1) * 5040],
                                     mybir.ActivationFunctionType.Exp)
                # S_dj[dl, il, h] = sum_di E[dj, di]
                e0 = E[:, dj * 5040:dj * 5040 + 1680]
                e1 = E[:, dj * 5040 + 1680:dj * 5040 + 3360]
                e2 = E[:, dj * 5040 + 3360:dj * 5040 + 5040]
                s1 = wk.tile([P_OUT, 3 * ILH * NH], fp16, name=f"s1_{dj}")
                nc.vector.scalar_tensor_tensor(s1[:], e0, 1.0, e1, MULT, ADD)
                sd = wk.tile([P_OUT, 3 * ILH * NH], fp16, name=f"sd{dj}")
                nc.vector.scalar_tensor_tensor(sd[:], s1[:], 1.0, e2,
                                               MULT, ADD)
                Sdj.append(sd)
                # T_dj[il, h] = sum_dl S_dj
                t1 = wk.tile([P_OUT, ILH * NH], fp16, name=f"t1_{dj}")
                nc.vector.scalar_tensor_tensor(
                    t1[:], sd[:, 0:560], 1.0, sd[:, 560:1120], MULT, ADD)
                td = wk.tile([P_OUT, ILH * NH], fp16, name=f"td{dj}")
                nc.vector.scalar_tensor_tensor(
                    td[:], t1[:], 1.0, sd[:, 1120:1680], MULT, ADD)
                Tdj.append(td)

            # denominator and the three signed numerators
            d1 = wk.tile([P_OUT, ILH * NH], fp16)
            nc.vector.scalar_tensor_tensor(d1[:], Tdj[0][:], 1.0, Tdj[1][:],
                                           MULT, ADD)
            denom = wk.tile([P_OUT, ILH * NH], fp16)
            nc.vector.scalar_tensor_tensor(denom[:], d1[:], 1.0, Tdj[2][:],
                                           MULT, ADD)
            # o=1 axis (j direction): T2 - T0
            nj = wk.tile([P_OUT, ILH * NH], fp16)
            nc.vector.scalar_tensor_tensor(nj[:], Tdj[2][:], 1.0, Tdj[0][:],
                                           MULT, SUB)
            # o=2 axis (l direction): sum_dj (S[dl=2] - S[dl=0])
            dls = []
            for dj in range(3):
                dd = wk.tile([P_OUT, ILH * NH], fp16, name=f"dd{dj}")
                nc.vector.scalar_tensor_tensor(
                    dd[:], Sdj[dj][:, 1120:1680], 1.0, Sdj[dj][:, 0:560],
                    MULT, SUB)
                dls.append(dd)
            nl1 = wk.tile([P_OUT, ILH * NH], fp16)
            nc.vector.scalar_tensor_tensor(nl1[:], dls[0][:], 1.0, dls[1][:],
                                           MULT, ADD)
            nl = wk.tile([P_OUT, ILH * NH], fp16)
            nc.vector.scalar_tensor_tensor(nl[:], nl1[:], 1.0, dls[2][:],
                                           MULT, ADD)
            # o=0 axis (i direction): sum_dj sum_dl (E[di=2] - E[di=0])
            vs = []
            for dj in range(3):
                vv = wk.tile([P_OUT, 3 * ILH * NH], fp16, name=f"vv{dj}")
                ep = E[:, dj * 5040 + 3360:dj * 5040 + 5040]
                em = E[:, dj * 5040:dj * 5040 + 1680]
                nc.vector.scalar_tensor_tensor(vv[:], ep, 1.0, em, MULT, SUB)
                vs.append(vv)
            v1 = wk.tile([P_OUT, 3 * ILH * NH], fp16)
            nc.vector.scalar_tensor_tensor(v1[:], vs[0][:], 1.0, vs[1][:],
                                           MULT, ADD)
            v2 = wk.tile([P_OUT, 3 * ILH * NH], fp16)
            nc.vector.scalar_tensor_tensor(v2[:], v1[:], 1.0, vs[2][:],
                                           MULT, ADD)
            ni1 = wk.tile([P_OUT, ILH * NH], fp16)
            nc.vector.scalar_tensor_tensor(
                ni1[:], v2[:, 0:560], 1.0, v2[:, 560:1120], MULT, ADD)
            ni = wk.tile([P_OUT, ILH * NH], fp16)
            nc.vector.scalar_tensor_tensor(ni[:], ni1[:], 1.0,
                                           v2[:, 1120:1680], MULT, ADD)

            # reciprocal (fp32 for accuracy), downcast on Act engine
            r32 = wk.tile([P_OUT, ILH * NH], fp32)
            nc.vector.reciprocal(r32[:], denom[:])
            r16 = wk.tile([P_OUT, ILH * NH], fp16)
            nc.scalar.copy(r16[:], r32[:])

            out = wk.tile([P_OUT, OF], fp16)
            for o, num in enumerate((ni, nj, nl)):
                nc.vector.scalar_tensor_tensor(
                    out[:, o * 560:(o + 1) * 560], num[:], 1.0, r16[:],
                    MULT, MULT)
            for c in range(2):
                nc.sync.dma_start(od[:, c * 840:(c + 1) * 840],
                                  out[:, c * 840:(c + 1) * 840])

    nc.compile()
    return nc


# channel permutation: on-chip c' = d*8 + h  <->  reference c = h*6 + d
_PERM = np.array([(c % 8) * 6 + c // 8 for c in range(DIM)])


def _host_prep(q, k, rpb):
    f16 = np.float16
    q0 = (np.asarray(q, np.float32)[0] * SCALE).astype(f16)[..., _PERM]
    k0 = np.asarray(k, np.float32)[0].astype(f16)[..., _PERM]
    # padded k: 1 voxel halo everywhere + 2 extra trailing l pads
    kp = np.pad(k0, ((1, 1), (1, 1), (1, 3), (0, 0)))     # [42,42,44,48]
    qp = np.pad(q0, ((0, 0), (0, 0), (0, 2), (0, 0)))     # [40,40,42,48]

    rpb16 = (np.asarray(rpb, np.float32) - SHIFT).astype(f16)  # [8,3,3,3]
    rpb_t = np.ascontiguousarray(
        rpb16.transpose(1, 2, 3, 0)).reshape(27 * NH)     # (di,dj,dl,h)
    rpb_b = np.broadcast_to(rpb_t, (P_OUT, 27 * NH)).copy()

    # q tile [120 = (j, lc), (i5, l14, c48)]
    # partition (j, lc) free (i, ll, c) = qp[i0+i, j, lc*14+ll, c]
    in_maps = []
    for core in range(N_CORES):
        i0 = core * SLAB
        qs = qp[i0:i0 + SLAB]                              # [5,40,42,48]
        # -> [j, lc, i, ll, c]
        qt = qs.reshape(SLAB, W, NLC, LCH, DIM).transpose(1, 2, 0, 3, 4)
        qt = np.ascontiguousarray(qt).reshape(P_OUT, QF)
        # k array [126 = (jp, lc), (i7, lk16, c48)]
        ks = kp[i0:i0 + KI]                                # [7,42,44,48]
        idx_l = (np.arange(NLC)[:, None] * LCH +
                 np.arange(KL)[None, :])                   # [3,16] l indices
        kk = ks[:, :, idx_l]                               # [7,42,3,16,48]
        kk = kk.transpose(1, 2, 0, 3, 4)                   # [42,3,7,16,48]
        kk = np.ascontiguousarray(kk).reshape(P_K, KF)
        in_maps.append({"qd": qt, "kd": kk, "rd": rpb_b})
    return in_maps


def _assemble(results):
    full = np.zeros((H, W, T, NH, 3), np.float32)
    for core in range(N_CORES):
        i0 = core * SLAB
        o = results[core]["od"].astype(np.float32)
        o = o.reshape(W, NLC, 3, SLAB, LCH, NH)  # [j, lc, o, i, ll, h]
        for lc in range(NLC):
            nl = LCH if lc < 2 else T - 2 * LCH
            full[i0:i0 + SLAB, :, lc * LCH:lc * LCH + nl] = (
                o[:, lc, :, :, :nl].transpose(2, 0, 3, 4, 1))
    # reference channel order c = h*3 + o
    out = full.reshape(H, W, T, NH * 3).transpose(3, 0, 1, 2)[None]
    return np.ascontiguousarray(out)


def _run(q, k, rpb, **spmd_kwargs):
    if "prog" not in _prog_cache:
        _prog_cache["prog"] = _build_program()
    nc = _prog_cache["prog"]
    in_maps = _host_prep(q, k, rpb)
    res = run_bass_kernel_spmd(nc, in_maps, list(range(N_CORES)),
                               **spmd_kwargs)
    return _assemble(res.results), res


def kernel(q, k, rpb):
    out, _ = _run(q, k, rpb)
    return out


# revision 7
# speedup vs baseline: 2.1613x; 1.8115x over previous
"""Trainium2 Bass kernel for 3D neighborhood attention — v3.

Same decomposition as v2 (see kernel.py docstring) but instruction mix
tuned to measured TRN2 DVE behavior:
  - two-tensor ops only via TENSOR_TENSOR (runs 2x with packed fp16;
    SCALAR_TENSOR_TENSOR measures 1x regardless of dtype),
  - per (dj,di) group: one coarse 4-dim-view product instruction and a
    4-instruction pairwise d-reduction tree (the rel-pos bias block,
    host-expanded to the full logits layout, enters as a tree leaf).
"""

import numpy as np

import concourse.bass as bass
import concourse.tile as tile
from concourse import bacc, mybir
from concourse.ap import AP
from concourse.bass_utils import run_bass_kernel_spmd

NH = 8
HD = 6
DIM = NH * HD
KS = 3
SCALE = HD**-0.5
SHIFT = 2.0
H = W = T = 40
N_CORES = 8
SLAB = H // N_CORES
NLC = 3
LCH = 14
P_OUT = W * NLC              # 120
P_K = (W + 2) * NLC          # 126
ILH = SLAB * LCH             # 70
QF = SLAB * LCH * DIM        # 3360
KI = SLAB + 2                # 7
KL = LCH + 2                 # 16
KF = KI * KL * DIM           # 5376
LF = 27 * ILH * NH           # 15120
DJF = 9 * ILH * NH           # 5040 per-dj logits block
OF = 3 * ILH * NH            # 1680

MULT = mybir.AluOpType.mult
ADD = mybir.AluOpType.add
SUB = mybir.AluOpType.subtract

_prog_cache = {}


def _ap(base, offset, dims):
    return AP(base.tensor, base.offset + offset,
              [list(base.ap[0])] + [list(d) for d in dims])


def _build_program():
    fp16 = mybir.dt.float16
    fp32 = mybir.dt.float32
    nc = bacc.Bacc("TRN2", target_bir_lowering=False, debug=False,
                   num_devices=N_CORES)
    qd = nc.dram_tensor("qd", [P_OUT, QF], fp16, kind="ExternalInput").ap()
    kd = nc.dram_tensor("kd", [P_K, KF], fp16, kind="ExternalInput").ap()
    rd = nc.dram_tensor("rd", [P_OUT, 27 * NH], fp16,
                        kind="ExternalInput").ap()
    od = nc.dram_tensor("od", [P_OUT, OF], fp16, kind="ExternalOutput").ap()

    def tt(out, in0, in1, op, acc=None):
        eng = nc.vector
        return eng.add_instruction(mybir.InstTensorTensor(
            name=nc.get_next_instruction_name(),
            op=op, acc=acc,
            ins=[eng.lower_ap(in0), eng.lower_ap(in1)],
            outs=[eng.lower_ap(out)],
        ))

    with tile.TileContext(nc) as tc:
        with (
            tc.tile_pool(name="io", bufs=1) as io,
            tc.tile_pool(name="work", bufs=1) as wk,
        ):
            rpb = io.tile([P_OUT, 27 * NH], fp16)
            nc.sync.dma_start(rpb[:], rd[:])
            q = io.tile([P_OUT, QF], fp16)
            for c in range(2):
                nc.sync.dma_start(q[:, c * 1680:(c + 1) * 1680],
                                  qd[:, c * 1680:(c + 1) * 1680])
            kt = []
            for dj in range(3):
                t = io.tile([P_OUT, KF], fp16, name=f"kt{dj}")
                for c in range(4):
                    nc.sync.dma_start(
                        t[:, c * 1344:(c + 1) * 1344],
                        kd[3 * dj:3 * dj + P_OUT, c * 1344:(c + 1) * 1344])
                kt.append(t)

            L = wk.tile([P_OUT, LF], fp16)   # (dj, di, dl, il, h)
            E = wk.tile([P_OUT, LF], fp16)
            Sdj = []
            Tdj = []

            qv4 = _ap(q[:], 0, [(0, 3), (672, 5), (48, 14), (1, 48)])
            for dj in range(3):
                for di in range(3):
                    # products for all (dl, d, h) of this (dj, di) in one
                    # TENSOR_TENSOR (2x with packed fp16); P4 = (dl, il, dh)
                    P4 = wk.tile([P_OUT, 3 * ILH * DIM], fp16)
                    kv4 = _ap(kt[dj][:], di * (KL * DIM),
                              [(48, 3), (768, 5), (48, 14), (1, 48)])
                    pv4 = _ap(P4[:], 0, [(3360, 3), (672, 5), (48, 14),
                                         (1, 48)])
                    tt(pv4, qv4, kv4, MULT)
                    # d-reduction tree: 6 -> 3 -> (+rpb) -> 1
                    A = wk.tile([P_OUT, 3 * ILH * 3 * NH], fp16)
                    av = _ap(A[:], 0, [(1680, 3), (24, 70), (8, 3), (1, 8)])
                    p0 = _ap(P4[:], 0, [(3360, 3), (48, 70), (8, 3), (1, 8)])
                    p1 = _ap(P4[:], 24, [(3360, 3), (48, 70), (8, 3), (1, 8)])
                    tt(av, p0, p1, ADD)
                    A2 = wk.tile([P_OUT, 3 * ILH * NH], fp16)
                    a0 = _ap(A[:], 0, [(1680, 3), (24, 70), (1, 8)])
                    a1 = _ap(A[:], 8, [(1680, 3), (24, 70), (1, 8)])
                    tt(A2[:], a0, a1, ADD)
                    A3 = wk.tile([P_OUT, 3 * ILH * NH], fp16)
                    a2 = _ap(A[:], 16, [(1680, 3), (24, 70), (1, 8)])
                    rv = _ap(rpb[:], di * 72 + dj * 24,
                             [(8, 3), (0, 70), (1, 8)])
                    tt(A3[:], a2, rv, ADD)
                    lv = _ap(L[:], dj * DJF + di * 1680,
                             [(560, 3), (8, 70), (1, 8)])
                    tt(lv, A2[:], A3[:], ADD)
                nc.scalar.activation(E[:, dj * DJF:(dj + 1) * DJF],
                                     L[:, dj * DJF:(dj + 1) * DJF],
                                     mybir.ActivationFunctionType.Exp)
                e0 = E[:, dj * DJF:dj * DJF + 1680]
                e1 = E[:, dj * DJF + 1680:dj * DJF + 3360]
                e2 = E[:, dj * DJF + 3360:dj * DJF + 5040]
                s1 = wk.tile([P_OUT, 1680], fp16, name=f"s1_{dj}")
                nc.vector.tensor_add(s1[:], e0, e1)
                sd = wk.tile([P_OUT, 1680], fp16, name=f"sd{dj}")
                nc.vector.tensor_add(sd[:], s1[:], e2)
                Sdj.append(sd)
                t1 = wk.tile([P_OUT, 560], fp16, name=f"t1_{dj}")
                nc.vector.tensor_add(t1[:], sd[:, 0:560], sd[:, 560:1120])
                td = wk.tile([P_OUT, 560], fp16, name=f"td{dj}")
                nc.vector.tensor_add(td[:], t1[:], sd[:, 1120:1680])
                Tdj.append(td)

            d1 = wk.tile([P_OUT, 560], fp16)
            nc.vector.tensor_add(d1[:], Tdj[0][:], Tdj[1][:])
            denom = wk.tile([P_OUT, 560], fp16)
            nc.vector.tensor_add(denom[:], d1[:], Tdj[2][:])
            nj = wk.tile([P_OUT, 560], fp16)
            nc.vector.tensor_sub(nj[:], Tdj[2][:], Tdj[0][:])
            dls = []
            for dj in range(3):
                dd = wk.tile([P_OUT, 560], fp16, name=f"dd{dj}")
                nc.vector.tensor_sub(dd[:], Sdj[dj][:, 1120:1680],
                                     Sdj[dj][:, 0:560])
                dls.append(dd)
            nl1 = wk.tile([P_OUT, 560], fp16)
            nc.vector.tensor_add(nl1[:], dls[0][:], dls[1][:])
            nl = wk.tile([P_OUT, 560], fp16)
            nc.vector.tensor_add(nl[:], nl1[:], dls[2][:])
            vs = []
            for dj in range(3):
                vv = wk.tile([P_OUT, 1680], fp16, name=f"vv{dj}")
                nc.vector.tensor_sub(vv[:],
                                     E[:, dj * DJF + 3360:dj * DJF + 5040],
                                     E[:, dj * DJF:dj * DJF + 1680])
                vs.append(vv)
            v1 = wk.tile([P_OUT, 1680], fp16)
            nc.vector.tensor_add(v1[:], vs[0][:], vs[1][:])
            v2 = wk.tile([P_OUT, 1680], fp16)
            nc.vector.tensor_add(v2[:], v1[:], vs[2][:])
            ni1 = wk.tile([P_OUT, 560], fp16)
            nc.vector.tensor_add(ni1[:], v2[:, 0:560], v2[:, 560:1120])
            ni = wk.tile([P_OUT, 560], fp16)
            nc.vector.tensor_add(ni[:], ni1[:], v2[:, 1120:1680])

            r32 = wk.tile([P_OUT, 560], fp32)
            nc.vector.reciprocal(r32[:], denom[:])
            r16 = wk.tile([P_OUT, 560], fp16)
            nc.scalar.copy(r16[:], r32[:])

            out = wk.tile([P_OUT, OF], fp16)
            for o, num in enumerate((ni, nj, nl)):
                nc.vector.tensor_mul(out[:, o * 560:(o + 1) * 560],
                                     num[:], r16[:])
            for c in range(2):
                nc.sync.dma_start(od[:, c * 840:(c + 1) * 840],
                                  out[:, c * 840:(c + 1) * 840])

    nc.compile()
    return nc


_PERM = np.array([(c % 8) * 6 + c // 8 for c in range(DIM)])


def _host_prep(q, k, rpb):
    f16 = np.float16
    q0 = (np.asarray(q, np.float32)[0] * SCALE).astype(f16)[..., _PERM]
    k0 = np.asarray(k, np.float32)[0].astype(f16)[..., _PERM]
    kp = np.pad(k0, ((1, 1), (1, 1), (1, 3), (0, 0)))     # [42,42,44,48]
    qp = np.pad(q0, ((0, 0), (0, 0), (0, 2), (0, 0)))     # [40,40,42,48]

    rpb16 = (np.asarray(rpb, np.float32) - SHIFT).astype(f16)  # [8,3,3,3]
    rpb_t = np.ascontiguousarray(
        rpb16.transpose(1, 2, 3, 0)).reshape(27 * NH)     # (di,dj,dl,h)
    rpbx = np.broadcast_to(rpb_t, (P_OUT, 27 * NH)).copy()

    in_maps = []
    for core in range(N_CORES):
        i0 = core * SLAB
        qs = qp[i0:i0 + SLAB]
        qt = qs.reshape(SLAB, W, NLC, LCH, DIM).transpose(1, 2, 0, 3, 4)
        qt = np.ascontiguousarray(qt).reshape(P_OUT, QF)
        ks = kp[i0:i0 + KI]
        idx_l = (np.arange(NLC)[:, None] * LCH + np.arange(KL)[None, :])
        kk = ks[:, :, idx_l]
        kk = kk.transpose(1, 2, 0, 3, 4)
        kk = np.ascontiguousarray(kk).reshape(P_K, KF)
        in_maps.append({"qd": qt, "kd": kk, "rd": rpbx})
    return in_maps


def _assemble(results):
    full = np.zeros((H, W, T, NH, 3), np.float32)
    for core in range(N_CORES):
        i0 = core * SLAB
        o = results[core]["od"].astype(np.float32)
        o = o.reshape(W, NLC, 3, SLAB, LCH, NH)
        for lc in range(NLC):
            nl_ = LCH if lc < 2 else T - 2 * LCH
            full[i0:i0 + SLAB, :, lc * LCH:lc * LCH + nl_] = (
                o[:, lc, :, :, :nl_].transpose(2, 0, 3, 4, 1))
    out = full.reshape(H, W, T, NH * 3).transpose(3, 0, 1, 2)[None]
    return np.ascontiguousarray(out)


def _run(q, k, rpb, **spmd_kwargs):
    if "prog" not in _prog_cache:
        _prog_cache["prog"] = _build_program()
    nc = _prog_cache["prog"]
    in_maps = _host_prep(q, k, rpb)
    res = run_bass_kernel_spmd(nc, in_maps, list(range(N_CORES)),
                               **spmd_kwargs)
    return _assemble(res.results), res


def kernel(q, k, rpb):
    out, _ = _run(q, k, rpb)
    return out
